# revision 23
# baseline (speedup 1.0000x reference)
"""Trainium2 Bass kernel for a single attention head (no softmax):

    q = x @ Wq + bq ; k = x @ Wk + bk ; v = x @ Wv + bv     [B,N,H]
    out = ((q @ k^T) * 768**-0.5) @ v                        [B,N,H]

Since there is no softmax between the two products, associativity gives

    out = q_s @ S,   S = k^T @ v  (64x64 per batch),  q_s = q * scale

which removes the O(N^2) score matrix entirely. Per-core work is just
the three projections over the core's own rows plus tiny 64x64 matmuls.

Sharding: 8 cores = 4 batches x 2 sequence halves. Core c handles batch
c//2, rows [h*2048, (h+1)*2048) with h = c%2 -- no duplication. The
partial S from each half is summed across the pair with one 16 KiB
AllReduce (replica groups [[0,1],[2,3],[4,5],[6,7]]).

Layout/dtype choices:
  - x arrives host-transposed+tiled (xp [4,128,6,512]) in bf16 (3 MiB
    per core; rel-err budget is 2e-2, bf16 costs ~0.4%).
  - projections: stationary [Wk|Wv] (M=128) and Wq*scale (M=64) against
    512-col x tiles; PSUM accumulates over the 6 e-chunks; ACT applies
    bias while casting to bf16.
  - kvT -> kv-natural via plain matmuls against the identity (a full-rate
    bf16 transpose; the dedicated PE transpose mode is ~3x slower and the
    DMA xbar transpose both underperforms and gets serialized against
    collectives by the scheduler). All 4 chunks of a tile share one PSUM
    bank, drained by one wide DVE copy.
  - S accumulates over 16 n-chunks of [128,64]x[128,64] matmuls.
  - out^T = S^T-contracted against qT: 4 matmuls [64,64]x[64,512],
    software-pipelined TWO iterations behind the projections so the PE
    never stalls on the AllReduce (which measurably blocks model DMA for
    ~5-6 us per invocation on this runtime).

Timing support: _get_program(loop_r) UNROLLS the body loop_r times
(collective_compute cannot execute inside a tc.For_i hardware loop on
this runtime; unrolled repetition works and lets iterations pipeline).
"""

import sys

sys.path.insert(0, "/opt/trn_rl_repo")

import numpy as np
import ml_dtypes

import concourse.bass as bass
import concourse.tile as tile
from concourse import bacc, mybir
from concourse.masks import make_identity

F32 = mybir.dt.float32
BF16 = mybir.dt.bfloat16
AF = mybir.ActivationFunctionType

B, N, E, H = 4, 4096, 768, 64
NCORES = 8
HALF = N // 2  # 2048 rows per core
TS = 512  # n-tile size
NT = HALF // TS  # 4 tiles per core
EC = E // 128  # 6 contraction chunks
CPT = TS // 128  # 4 n-chunks (of 128) per tile
NCHUNK = HALF // 128  # 16 n-chunks per core
SCALE = np.float32(1.0) / np.sqrt(np.float32(E))
GROUPS = [[0, 1], [2, 3], [4, 5], [6, 7]]

_cache = {}


def _build_program(loop_r=1):
    nc = bacc.Bacc(None, num_devices=NCORES)
    xp = nc.declare_dram_parameter("xp", [NT, 128, EC, TS], BF16, isOutput=False)
    wkv = nc.declare_dram_parameter("wkv", [128, EC, 128], BF16, isOutput=False)
    wq = nc.declare_dram_parameter("wq", [128, EC, H], BF16, isOutput=False)
    bkv = nc.declare_dram_parameter("bkv", [128, 1], F32, isOutput=False)
    bq = nc.declare_dram_parameter("bq", [H, 1], F32, isOutput=False)
    out = nc.declare_dram_parameter("out", [H, HALF], BF16, isOutput=True)

    with tile.TileContext(nc) as tc:
        with (
            tc.tile_pool(name="const", bufs=1) as const,
            tc.tile_pool(name="bigp", bufs=4) as bigp,
            tc.tile_pool(name="xtp", bufs=4) as xtp,
            tc.tile_pool(name="smsb", bufs=4) as smsb,
            tc.tile_pool(name="tmpp", bufs=3) as tmpp,
            tc.tile_pool(name="projp", bufs=3, space="PSUM") as projp,
            tc.tile_pool(name="qpp", bufs=2, space="PSUM") as qpp,
            tc.tile_pool(name="spp", bufs=1, space="PSUM") as spp,
            tc.tile_pool(name="opp", bufs=1, space="PSUM") as opp,
            tc.tile_pool(name="trp", bufs=1, space="PSUM") as trp,
            tc.tile_pool(name="dram", bufs=4, space="DRAM") as dram,
        ):
            wkv_t = const.tile([128, EC, 128], BF16)
            wq_t = const.tile([128, EC, H], BF16)
            bkv_t = const.tile([128, 1], F32)
            bq_t = const.tile([H, 1], F32)
            ident_f = const.tile([128, 128], F32)
            ident = const.tile([128, 128], BF16)
            nc.sync.dma_start(wkv_t[:], wkv[:])
            nc.sync.dma_start(wq_t[:], wq[:])
            nc.sync.dma_start(bkv_t[:], bkv[:])
            nc.sync.dma_start(bq_t[:], bq[:])
            make_identity(nc, ident_f[:])
            nc.vector.tensor_copy(ident[:], ident_f[:])

            # DMA ring assignment (each HWDGE ring is FIFO per issuing
            # engine, so a blocked DMA head-of-line blocks the ring):
            #   nc.sync (SP ring):    xp tile loads only -> free prefetch
            #   nc.scalar (ACT ring): out store
            #   nc.gpsimd (SWDGE):    collective bounce DMAs
            pending = []  # (s2, qT, outT) awaiting out-block, 2-iter delay

            def out_block(p):
                s2_p, qT_p, outT_p = p
                for qq in range(NT):
                    qcols = slice(qq * TS, (qq + 1) * TS)
                    o_ps = opp.tile([H, TS], F32, tag="o")
                    nc.tensor.matmul(
                        o_ps[:], s2_p[:], qT_p[:, qcols], start=True, stop=True
                    )
                    nc.vector.tensor_copy(outT_p[:, qcols], o_ps[:])
                nc.scalar.dma_start(out[:], outT_p[:])

            for _ in range(loop_r):
                kvT = bigp.tile([128, HALF], BF16, tag="kvT")
                qT = bigp.tile([H, HALF], BF16, tag="qT")
                kvnat = bigp.tile([128, NCHUNK, 128], BF16, tag="kvnat")
                outT = bigp.tile([H, HALF], BF16, tag="outT")
                s_ps = spp.tile([H, H], F32, tag="s")

                def s_chunks(t):
                    for u in range(t * CPT, (t + 1) * CPT):
                        nc.tensor.matmul(
                            s_ps[:],
                            kvnat[:, u, 0:H],
                            kvnat[:, u, H:128],
                            start=(u == 0),
                            stop=(u == NCHUNK - 1),
                            skip_group_check=True,
                        )

                for t in range(NT):
                    xt = xtp.tile([128, EC, TS], BF16, tag="xt")
                    nc.sync.dma_start(xt[:], xp[t])
                    cols = slice(t * TS, (t + 1) * TS)

                    kv_ps = projp.tile([128, TS], F32, tag="kv")
                    for cc in range(EC):
                        nc.tensor.matmul(
                            kv_ps[:],
                            wkv_t[:, cc, :],
                            xt[:, cc, :],
                            start=(cc == 0),
                            stop=(cc == EC - 1),
                        )
                    # q projection: col-tiled pairs, even e-chunks on col
                    # groups 0-1 (psum partitions 0:64), odd on 2-3 (64:128)
                    q_ps = qpp.tile([128, TS], F32, tag="q")
                    for cc in range(0, EC, 2):
                        nc.tensor.matmul(
                            q_ps[0:H, :],
                            wq_t[:, cc, :],
                            xt[:, cc, :],
                            start=(cc == 0),
                            stop=(cc == EC - 2),
                            tile_position=(0, 0),
                            skip_group_check=True,
                        )
                        nc.tensor.matmul(
                            q_ps[H:128, :],
                            wq_t[:, cc + 1, :],
                            xt[:, cc + 1, :],
                            start=(cc == 0),
                            stop=(cc == EC - 2),
                            tile_position=(0, H),
                            skip_group_check=True,
                        )
                    # S-chunk matmuls of the previous tile slot in after
                    # this tile's projections (transpose DMA has landed)
                    if t >= 1:
                        s_chunks(t - 1)

                    nc.scalar.activation(
                        kvT[:, cols], kv_ps[:], AF.Identity, bias=bkv_t[:]
                    )
                    qtmp = tmpp.tile([H, TS], F32, tag="qtmp")
                    nc.scalar.activation(qtmp[:], q_ps[0:H, :], AF.Identity, bias=bq_t[:])
                    nc.vector.tensor_add(qT[:, cols], qtmp[:], q_ps[H:128, :])
                    # Transpose this tile's 4 [128,128] kvT chunks into
                    # natural [n, k|v] layout for the S matmuls. A plain
                    # matmul against the identity IS the transpose
                    # (out = chunk^T @ I); all 4 land in one PSUM bank and
                    # one wide DVE copy moves them to SBUF.
                    tr_ps = trp.tile([128, 4, 128], F32, tag="tr")
                    for ui, u in enumerate(range(t * CPT, (t + 1) * CPT)):
                        nc.tensor.matmul(
                            tr_ps[:, ui, :],
                            kvT[:, u * 128 : (u + 1) * 128],
                            ident[:],
                            start=True,
                            stop=True,
                            skip_group_check=True,
                        )
                    nc.vector.tensor_copy(
                        kvnat[:, t * CPT : (t + 1) * CPT, :], tr_ps[:]
                    )
                s_chunks(NT - 1)

                s_sb = smsb.tile([H, H], BF16, tag="s_sb")
                nc.vector.tensor_copy(s_sb[:], s_ps[:])
                ib = dram.tile([H, H], BF16, tag="ib")
                ob = dram.tile([H, H], BF16, tag="ob")
                nc.gpsimd.dma_start(ib[:], s_sb[:])
                nc.gpsimd.collective_compute(
                    "AllReduce",
                    mybir.AluOpType.add,
                    replica_groups=GROUPS,
                    ins=[ib.opt()],
                    outs=[ob.opt()],
                )
                s2 = smsb.tile([H, H], BF16, tag="s2")
                nc.gpsimd.dma_start(s2[:], ob[:])

                # out-blocks run TWO iterations late so the PE queue never
                # stalls on a collective still in flight
                pending.append((s2, qT, outT))
                if len(pending) > 2:
                    out_block(pending.pop(0))
            for p in pending:
                out_block(p)

    nc.compile()
    return nc


def _prep_inputs(x, Wq, bq, Wk, bk, Wv, bv):
    x = np.asarray(x, dtype=np.float32)
    Wq = np.asarray(Wq, dtype=np.float32)
    Wk = np.asarray(Wk, dtype=np.float32)
    Wv = np.asarray(Wv, dtype=np.float32)
    bq = np.asarray(bq, dtype=np.float32)
    bk = np.asarray(bk, dtype=np.float32)
    bv = np.asarray(bv, dtype=np.float32)

    def prep_w(w):  # [768, M] -> [128, 6, M]
        return np.ascontiguousarray(
            w.reshape(EC, 128, w.shape[1]).transpose(1, 0, 2).astype(ml_dtypes.bfloat16)
        )

    wkv_p = prep_w(np.concatenate([Wk, Wv], axis=1))
    wq_p = prep_w(Wq * SCALE)
    bkv_p = np.ascontiguousarray(np.concatenate([bk, bv]).reshape(128, 1))
    bq_p = np.ascontiguousarray((bq * SCALE).reshape(H, 1))

    in_maps = []
    for c in range(NCORES):
        b, h = divmod(c, 2)
        own = x[b, h * HALF : (h + 1) * HALF]  # [2048, 768]
        xp = np.ascontiguousarray(
            own.reshape(NT, TS, EC, 128).transpose(0, 3, 2, 1).astype(ml_dtypes.bfloat16)
        )  # [4, 128, 6, 512]
        in_maps.append(
            {"xp": xp, "wkv": wkv_p, "wq": wq_p, "bkv": bkv_p, "bq": bq_p}
        )
    return in_maps


def _get_program(loop_r=1):
    key = ("nc", loop_r)
    if key not in _cache:
        _cache[key] = _build_program(loop_r)
    return _cache[key]


def _run_spmd_once(in_maps):
    from concourse.bass_utils import run_bass_kernel_spmd

    nc = _get_program()
    return run_bass_kernel_spmd(nc, in_maps, list(range(NCORES))).results


def _build_fast_runner():
    """jit the SPMD dispatch once so repeated kernel() calls skip
    re-tracing."""
    import jax
    from jax.sharding import Mesh, PartitionSpec
    from jax.experimental.shard_map import shard_map
    from concourse.bass2jax import (
        _bass_exec_p,
        install_neuronx_cc_hook,
        partition_id_tensor,
    )

    install_neuronx_cc_hook()
    nc = _get_program()
    partition_name = nc.partition_id_tensor.name if nc.partition_id_tensor else None

    in_names, out_names, out_avals, zero_outs = [], [], [], []
    for alloc in nc.m.functions[0].allocations:
        if not isinstance(alloc, mybir.MemoryLocationSet):
            continue
        name = alloc.memorylocations[0].name
        if alloc.kind == "ExternalInput":
            if name != partition_name:
                in_names.append(name)
        elif alloc.kind == "ExternalOutput":
            out_names.append(name)
            shape = tuple(alloc.tensor_shape)
            dtype = mybir.dt.np(alloc.dtype)
            out_avals.append(jax.core.ShapedArray(shape, dtype))
            zero_outs.append(np.zeros(shape, dtype))
    n_params = len(in_names)
    all_in_names = list(in_names) + list(out_names)
    if partition_name is not None:
        all_in_names = all_in_names + [partition_name]

    def _body(*args):
        operands = list(args)
        if partition_name is not None:
            operands.append(partition_id_tensor())
        outs = _bass_exec_p.bind(
            *operands,
            out_avals=tuple(out_avals),
            in_names=tuple(all_in_names),
            out_names=tuple(out_names),
            lowering_input_output_aliases=(),
            sim_require_finite=True,
            sim_require_nnan=True,
            nc=nc,
        )
        return tuple(outs)

    devices = jax.devices()[:NCORES]
    mesh = Mesh(np.asarray(devices), ("core",))
    in_specs = (PartitionSpec("core"),) * (n_params + len(out_names))
    out_specs = (PartitionSpec("core"),) * len(out_names)
    f = jax.jit(
        shard_map(
            _body, mesh=mesh, in_specs=in_specs, out_specs=out_specs,
            check_rep=False,
        ),
        keep_unused=True,
    )
    concat_zeros = [
        np.zeros((NCORES * z.shape[0],) + z.shape[1:], z.dtype) for z in zero_outs
    ]

    def run(in_maps):
        concat_in = [
            np.concatenate([np.asarray(in_maps[c][k]) for c in range(NCORES)], axis=0)
            for k in in_names
        ]
        out_arrs = f(*concat_in, *concat_zeros)
        return [
            {
                name: np.asarray(out_arrs[i]).reshape(NCORES, *out_avals[i].shape)[c]
                for i, name in enumerate(out_names)
            }
            for c in range(NCORES)
        ]

    return run


def _run(in_maps):
    if "fast_runner" not in _cache:
        _cache["fast_runner"] = _build_fast_runner()
    return _cache["fast_runner"](in_maps)


def _assemble(results):
    full = np.empty((B, N, H), dtype=np.float32)
    for c in range(NCORES):
        b, h = divmod(c, 2)
        full[b, h * HALF : (h + 1) * HALF, :] = (
            results[c]["out"].astype(np.float32).T
        )
    return full


def kernel(x, Wq, bq, Wk, bk, Wv, bv):
    in_maps = _prep_inputs(x, Wq, bq, Wk, bk, Wv, bv)
    res = _run(in_maps)
    return _assemble(res)
